# revision 48
# baseline (speedup 1.0000x reference)
"""Trainium2 Bass kernel for windowed multi-agent attention (Swin-style).

Full-input contract: kernel(**inputs) takes the unsharded inputs and returns
the unsharded output. Shards over the H axis across 8 NeuronCores; each core
handles 16 H-rows = 4 strips x 32 windows.

Design (per core):
  - Host pre-orders tokens into window order (f16); tokens DMA straight into
    SBUF (no on-device reorder; 512B+ descriptors; 30 DMAs total per core vs
    256 in the old version, fixing the HWDGE serialization bottleneck).
  - Q/K/V and the output projection are f16 matmuls; sim = q^T k per
    (window, head) in bf16, heads pair-stacked on partitions.
  - softmax: Act exp (psum->SBUF), DVE bias-mul / rowsum / reciprocal /
    normalize (free-axis ops).
  - normalized attention is PE-transposed per 64x64 block (bf16 psum) so AV
    contracts k on partitions and emits o^T (feature-major) directly - no
    second transpose of the output.
  - NOTE: 64-wide PE tiles must keep lhsT/rhs/out on the same partition half
    (strictly diagonal); mixing diagonal and cross tile positions on one psum
    tile wedges the device. V is therefore computed per (window, head) block
    with regular (non-DoubleRow) matmuls, since DoubleRow cannot place its
    destination at partition base 64.
  - output staged f16; host converts back to f32 NCHW.
"""

import numpy as np

HEADS = 4
WIN = 4
MAX_N = 5
DIM = 256
N_AGENTS = 4
H = W = 128
N_CORES = 8
HS = 16                 # H rows per core
N_STRIPS = 4            # strips per core (4 rows each)
N_GROUPS = 4            # groups of 8 windows per strip
GW = 8                  # windows per group
T = 64                  # tokens per window
NT = GW * T             # tokens per group = 512
ST = 4 * NT             # tokens per strip = 2048

SW = 64.0               # fp8 scale for wq/wk
SV = 16.0               # fp8 scale for wv
SO = 16.0               # fp8 scale for wo


def _rel_pos_index(N, wh, ww, md, mh, mw):
    cd, ch, cw = np.arange(N), np.arange(wh), np.arange(ww)
    coords = np.stack(np.meshgrid(cd, ch, cw, indexing="ij")).reshape(3, -1)
    rel = (coords[:, :, None] - coords[:, None, :]).transpose(1, 2, 0).astype(np.int64)
    rel[..., 0] += md - 1
    rel[..., 1] += mh - 1
    rel[..., 2] += mw - 1
    rel[..., 0] *= (2 * mh - 1) * (2 * mw - 1)
    rel[..., 1] *= 2 * mw - 1
    return rel.sum(-1)


def _build_bias(bias_table):
    """[128, 1024] f32: stacks A|B; rows (hh, q), cols (w8, k); exp'd."""
    rpi = _rel_pos_index(MAX_N, WIN, WIN, MAX_N, WIN, WIN)
    b = bias_table[rpi][:T, :T].transpose(2, 0, 1).astype(np.float32)  # (4, 64, 64) [h, q, k]
    out = np.zeros((128, 1024), np.float32)
    for st in range(2):
        for hh in range(2):
            blk = np.exp(b[st * 2 + hh])                 # (64 q, 64 k)
            out[hh * 64:(hh + 1) * 64, st * 512:(st + 1) * 512] = np.tile(blk, (1, GW))
    return np.ascontiguousarray(out)


def _patch_tile_drain():
    from concourse import tile as tile_mod
    from concourse.vector_clock import ScopedClock, VectorClock
    if getattr(tile_mod.TileContext, "_drain_patched", False):
        return

    def _patched(self, tick_clock, wait_clock):
        gc_ = tick_clock.global_clock
        n = len(gc_)
        for proc in range(n):
            tick = gc_[proc]
            if tick <= 0:
                continue
            vc = VectorClock([0] * n)
            vc.require_at_least(proc, tick)
            nop_inst = self.nc.sync.nop(nofuse=True)
            wait_clock.add_sem_waits(nop_inst.ins, ScopedClock({None: vc}))
        self.nc.sync.drain()
        self.nc.all_engine_barrier()
        popped = self.nc._tile_sem_poison_stack.pop()
        assert popped is self._sem_poison
        self.nc.clear_and_free_semaphores(list(self.sems.allocated().values()))
        self.nc.all_engine_barrier()

    tile_mod.TileContext._drain_and_barrier = _patched
    tile_mod.TileContext._drain_patched = True


def _split_multi_waits(nc):
    import orjson
    orig = nc.to_json_bytes

    def patched():
        bj = orjson.loads(orig())
        counter = [0]
        for fn in bj.get("functions", []):
            for blk in fn.get("blocks", []):
                insts = blk.get("instructions", [])
                out = []
                for inst in insts:
                    si = inst.get("sync_info") or {}
                    waits = si.get("on_wait") or []
                    if len(waits) > 1:
                        for w in waits[:-1]:
                            counter[0] += 1
                            out.append({
                                "name": f"WSPL-{counter[0]}",
                                "opcode": "NoOp",
                                "engine": inst["engine"],
                                "ins": [],
                                "outs": [],
                                "sync_info": {"on_update": [], "on_wait": [w]},
                            })
                        si["on_wait"] = [waits[-1]]
                    out.append(inst)
                blk["instructions"] = out
        return orjson.dumps(bj)

    nc.to_json_bytes = patched
    return nc


def build_nc():
    import os
    from concourse import bass, mybir
    from concourse.tile import TileContext
    from concourse.alu_op_type import AluOpType
    from contextlib import ExitStack
    _patch_tile_drain()
    STAGE = os.environ.get("KSTAGE", "full")
    OTDT = os.environ.get("KOTDT", "f8")
    OTSKIP = os.environ.get("KOTSKIP", "")

    def stage_ge(t):
        order = ["qkv", "v", "sim", "soft", "et", "avmm", "av", "ot", "full"]
        return order.index(STAGE) >= order.index(t)

    F32 = mybir.dt.float32
    F16 = mybir.dt.float16
    BF16 = mybir.dt.bfloat16
    F8 = mybir.dt.float8e4
    EXP = mybir.ActivationFunctionType.Exp
    IDENT = mybir.ActivationFunctionType.Copy
    DR = mybir.MatmulPerfMode.DoubleRow
    AX = mybir.AxisListType.X

    nc = bass.Bass("TRN2", target_bir_lowering=False, debug=False,
                   num_devices=N_CORES)

    xt_d = nc.dram_tensor("xt", [N_STRIPS, 128, 2, ST], F16, kind="ExternalInput").ap()
    wq_d = nc.dram_tensor("wq", [128, 2, 256], F16, kind="ExternalInput").ap()
    wk_d = nc.dram_tensor("wk", [128, 2, 256], F16, kind="ExternalInput").ap()
    wv_d = nc.dram_tensor("wv", [128, 2, 256], F16, kind="ExternalInput").ap()
    wo_d = nc.dram_tensor("wo", [128, 2, 256], F16, kind="ExternalInput").ap()
    bm_d = nc.dram_tensor("bm", [128, 1024], F32, kind="ExternalInput").ap()
    id_d = nc.dram_tensor("ident", [128, 128], F32, kind="ExternalInput").ap()
    out_d = nc.dram_tensor("out", [N_STRIPS, 128, 2 * ST], F16, kind="ExternalOutput").ap()

    with TileContext(nc) as tc, ExitStack() as stk, \
            nc.allow_low_precision(reason="bf16/fp8 attention"):
        cpool = stk.enter_context(tc.tile_pool(name="consts", bufs=1))
        wq = cpool.tile([128, 2, 256], F16, name="wq", tag="wq")
        wk = cpool.tile([128, 2, 256], F16, name="wk", tag="wk")
        wv = cpool.tile([128, 2, 256], F16, name="wv", tag="wv")
        wo = cpool.tile([128, 2, 256], F16, name="wo", tag="wo")
        bm = cpool.tile([128, 1024], BF16, name="bm", tag="bm")
        ident = cpool.tile([128, 128], BF16, name="ident", tag="ident")
        TT = [cpool.tile([128, 2, ST], F16, name=f"tt{s}", tag=f"tt{s}")
              for s in range(N_STRIPS)]
        # f16 weights need no cast: use the fast HWDGE queue, ahead of tokens
        nc.sync.dma_start(out=wq[:], in_=wq_d)
        nc.sync.dma_start(out=wk[:], in_=wk_d)
        nc.sync.dma_start(out=wv[:], in_=wv_d)
        nc.sync.dma_start(out=wo[:], in_=wo_d)
        nc.gpsimd.dma_start(out=bm[:], in_=bm_d)
        nc.gpsimd.dma_start(out=ident[:], in_=id_d)
        for gch in range(N_GROUPS):
            nc.sync.dma_start(out=TT[0][:, :, gch * NT:(gch + 1) * NT],
                              in_=xt_d[0][:, :, gch * NT:(gch + 1) * NT])
        for s in range(1, N_STRIPS):
            nc.sync.dma_start(out=TT[s][:], in_=xt_d[s])

        ospool = stk.enter_context(tc.tile_pool(name="os", bufs=2))
        grp = stk.enter_context(tc.tile_pool(name="grp", bufs=2))
        psA = stk.enter_context(tc.tile_pool(name="psA", bufs=2, space="PSUM"))
        psB = stk.enter_context(tc.tile_pool(name="psB", bufs=2, space="PSUM"))
        psC = stk.enter_context(tc.tile_pool(name="psC", bufs=2, space="PSUM"))
        psD = stk.enter_context(tc.tile_pool(name="psD", bufs=2, space="PSUM"))

        for s in range(N_STRIPS):
            OS = ospool.tile([128, 2 * ST], F16, name="OS", tag="OS")
            for g in range(N_GROUPS):
                gsl = slice(g * NT, (g + 1) * NT)

                # ---- Q/K projections: fp8 DoubleRow, contraction 256 ----
                QA = psA.tile([128, NT], F32, name="QA", tag="psA")
                QB = psA.tile([128, NT], F32, name="QB", tag="psA")
                KA = psA.tile([128, NT], F32, name="KA", tag="psA")
                KB = psA.tile([128, NT], F32, name="KB", tag="psA")
                for c in range(2):
                    st_, sp_ = (c == 0), (c == 1)
                    rhs_tok = TT[s][:, c, gsl]
                    nc.tensor.matmul(QA[:], wq[:, c, 0:128], rhs_tok, start=st_, stop=sp_)
                    nc.tensor.matmul(KA[:], wk[:, c, 0:128], rhs_tok, start=st_, stop=sp_)
                    nc.tensor.matmul(QB[:], wq[:, c, 128:256], rhs_tok, start=st_, stop=sp_)
                    nc.tensor.matmul(KB[:], wk[:, c, 128:256], rhs_tok, start=st_, stop=sp_)
                qA = grp.tile([128, NT], BF16, name="qA", tag="qA")
                kA = grp.tile([128, NT], BF16, name="kA", tag="kA")
                qB = grp.tile([128, NT], BF16, name="qB", tag="qB")
                kB = grp.tile([128, NT], BF16, name="kB", tag="kB")
                nc.scalar.activation(qA[:], QA[:], IDENT)
                nc.scalar.activation(kA[:], KA[:], IDENT)
                nc.scalar.activation(qB[:], QB[:], IDENT)
                nc.scalar.activation(kB[:], KB[:], IDENT)
                if not stage_ge("v"):
                    nc.scalar.activation(OS[:, g * NT:(g + 1) * NT], qA[:], IDENT)
                    nc.vector.tensor_copy(OS[:, ST + g * NT: ST + (g + 1) * NT], kB[:])
                    continue

                # ---- V projection: per (window, head) blocks into (hh, k)-row
                # layout so AV can run strictly diagonal. 128-contraction
                # matmuls may place output at any partition base. ----
                VP = [psB.tile([128, NT], F32, name=f"VP{j}", tag="psB") for j in range(2)]
                for st in range(2):
                    for w in range(GW):
                        for hh in range(2):
                            h = st * 2 + hh
                            for c in range(2):
                                nc.tensor.matmul(
                                    VP[st][hh * 64:(hh + 1) * 64, w * T:(w + 1) * T],
                                    TT[s][:, c, g * NT + w * T: g * NT + (w + 1) * T],
                                    wv[:, c, h * 64:(h + 1) * 64],
                                    start=(c == 0), stop=(c == 1))
                vSB = [grp.tile([128, NT], BF16, name=f"v{j}", tag=f"v{j}") for j in range(2)]
                nc.scalar.activation(vSB[0][:], VP[0][:], IDENT)
                nc.vector.tensor_copy(vSB[1][:], VP[1][:])
                if not stage_ge("sim"):
                    nc.scalar.activation(OS[:, g * NT:(g + 1) * NT], vSB[0][:], IDENT)
                    nc.vector.tensor_copy(OS[:, ST + g * NT: ST + (g + 1) * NT], vSB[1][:])
                    continue

                # ---- sim = q^T k per (window, head): rows (hh, q), cols (w, k) ----
                SS = [psC.tile([128, NT], F32, name=f"S{st}", tag="psC") for st in range(2)]
                for st, (qX, kX) in enumerate(((qA, kA), (qB, kB))):
                    for w in range(GW):
                        wt = slice(w * T, (w + 1) * T)
                        for hh in range(2):
                            pp = slice(hh * 64, (hh + 1) * 64)
                            nc.tensor.matmul(SS[st][pp, wt], qX[pp, wt], kX[pp, wt],
                                             start=True, stop=True)

                if not stage_ge("soft"):
                    nc.scalar.activation(OS[:, g * NT:(g + 1) * NT], SS[0][:], IDENT)
                    nc.vector.tensor_copy(OS[:, ST + g * NT: ST + (g + 1) * NT], SS[1][:])
                    continue
                # ---- softmax over k (free axis) ----
                EE = [grp.tile([128, NT], BF16, name=f"E{st}", tag=f"E{st}") for st in range(2)]
                rs = [grp.tile([128, GW], F32, name=f"rs{st}", tag=f"rs{st}") for st in range(2)]
                rr = [grp.tile([128, GW], BF16, name=f"rr{st}", tag=f"rr{st}") for st in range(2)]
                for st in range(2):
                    nc.scalar.activation(EE[st][:], SS[st][:], EXP)
                    nc.vector.tensor_mul(EE[st][:], EE[st][:], bm[:, st * 512:(st + 1) * 512])
                    nc.vector.reduce_sum(rs[st][:], EE[st][:].rearrange("p (w k) -> p w k", w=GW), axis=AX)
                    nc.vector.reciprocal(rr[st][:], rs[st][:])
                    nc.vector.tensor_mul(
                        EE[st][:].rearrange("p (w k) -> p w k", w=GW),
                        EE[st][:].rearrange("p (w k) -> p w k", w=GW),
                        rr[st][:].unsqueeze(2).broadcast_to([128, GW, T]))

                if not stage_ge("et"):
                    nc.scalar.activation(OS[:, g * NT:(g + 1) * NT], EE[0][:], IDENT)
                    nc.vector.tensor_copy(OS[:, ST + g * NT: ST + (g + 1) * NT], EE[1][:])
                    continue
                # ---- transpose E per (window, head) 64x64 block (diagonal):
                # rows (hh, k), cols (w, q) ----
                ETsup = psC.tile([128, 1024], BF16, name="ETsup", tag="psC")
                ET = [ETsup[:, 0:NT], ETsup[:, NT:2 * NT]]
                ETs = [grp.tile([128, NT], BF16, name=f"ETs{st}", tag=f"ETs{st}") for st in range(2)]
                for st in range(2):
                    for w in range(GW):
                        wt = slice(w * T, (w + 1) * T)
                        for hh in range(2):
                            pp = slice(hh * 64, (hh + 1) * 64)
                            nc.tensor.transpose(ET[st][pp.start:pp.stop, wt.start:wt.stop], EE[st][pp, wt], ident[pp, pp])
                    nc.vector.tensor_copy(ETs[st][:], ET[st])

                if not stage_ge("avmm"):
                    nc.scalar.activation(OS[:, g * NT:(g + 1) * NT], ETs[0][:], IDENT)
                    nc.vector.tensor_copy(OS[:, ST + g * NT: ST + (g + 1) * NT], ETs[1][:])
                    continue
                # ---- AV: o^T[d, q], rows (hh, d), cols (w, q); all diagonal ----
                OT = [psD.tile([128, NT], F32, name=f"OT{st}", tag="psD") for st in range(2)]
                for st in range(2):
                    for w in range(GW):
                        wt = slice(w * T, (w + 1) * T)
                        for hh in range(2):
                            pp = slice(hh * 64, (hh + 1) * 64)
                            nc.tensor.matmul(OT[st][pp, wt], vSB[st][pp, wt],
                                             ETs[st][pp, wt], start=True, stop=True)
                if not stage_ge("av"):
                    nc.scalar.activation(OS[:, g * NT:(g + 1) * NT], OT[0][:], IDENT)
                    nc.vector.tensor_copy(OS[:, ST + g * NT: ST + (g + 1) * NT], OT[1][:])
                    continue
                oT = [grp.tile([128, NT], F16, name=f"oT{c}", tag=f"oT{c}") for c in range(2)]
                nc.scalar.activation(oT[0][:], OT[0][:], IDENT)
                nc.vector.tensor_copy(oT[1][:], OT[1][:])

                if not stage_ge("ot"):
                    nc.scalar.activation(OS[:, g * NT:(g + 1) * NT], OT[0][:], IDENT)
                    if os.environ.get("KDUP", "") == "1":
                        nc.scalar.activation(OS[:, g * NT:(g + 1) * NT], OT[0][:], IDENT)
                    nc.vector.tensor_copy(OS[:, ST + g * NT: ST + (g + 1) * NT], OT[1][:])
                    continue
                if not stage_ge("full"):
                    nc.scalar.activation(OS[:, g * NT:(g + 1) * NT], oT[0][:], IDENT)
                    nc.vector.tensor_copy(OS[:, ST + g * NT: ST + (g + 1) * NT], oT[1][:])
                    continue
                # ---- output projection: fp8 DoubleRow over cin=256 ----
                UU = [psD.tile([128, NT], F32, name=f"U{c}", tag="psD") for c in range(2)]
                for c in range(2):
                    st_, sp_ = (c == 0), (c == 1)
                    rhs_o = oT[c][:]
                    nc.tensor.matmul(UU[0][:], wo[:, c, 0:128], rhs_o, start=st_, stop=sp_)
                    nc.tensor.matmul(UU[1][:], wo[:, c, 128:256], rhs_o, start=st_, stop=sp_)
                nc.scalar.activation(OS[:, g * NT:(g + 1) * NT], UU[0][:], IDENT)
                nc.vector.tensor_copy(OS[:, ST + g * NT: ST + (g + 1) * NT], UU[1][:])

            if s == N_STRIPS - 1:
                for gch in range(N_GROUPS):
                    nc.sync.dma_start(out=out_d[s][:, gch * NT: (gch + 1) * NT], in_=OS[:, gch * NT:(gch + 1) * NT])
                    nc.sync.dma_start(out=out_d[s][:, ST + gch * NT: ST + (gch + 1) * NT], in_=OS[:, ST + gch * NT: ST + (gch + 1) * NT])
            else:
                nc.sync.dma_start(out=out_d[s], in_=OS[:])

    return _split_multi_waits(nc)


_NC_CACHE = None


def _prep_inputs(x, w_qkv, w_out, bias_table):
    import ml_dtypes
    F8NP = ml_dtypes.float8_e4m3

    x = np.asarray(x, dtype=np.float32)
    w_qkv = np.asarray(w_qkv, dtype=np.float32)
    w_out = np.asarray(w_out, dtype=np.float32)
    bias_table = np.asarray(bias_table, dtype=np.float32)

    scale = (DIM // HEADS) ** -0.5

    def pack_w(wm, s):
        # [256 cin, 256 cout] -> [128, 2, 256] fp8 (cin = c*128+p)
        t = (wm * s).reshape(2, 128, 256).transpose(1, 0, 2)
        return np.ascontiguousarray(t).astype(F8NP)

    def pack_w16(wm):
        t = wm.reshape(2, 128, 256).transpose(1, 0, 2)
        return np.ascontiguousarray(t).astype(np.float16)

    wq = pack_w16(w_qkv[:, 0:DIM] * scale)
    wk = pack_w16(w_qkv[:, DIM:2 * DIM])
    wv = pack_w16(w_qkv[:, 2 * DIM:3 * DIM])
    wo = pack_w16(w_out)
    bmz = _build_bias(bias_table)
    identity = np.eye(128, dtype=np.float32)

    # tokens: per core m, XT [4 strips, 128 ch-part, 2 c, 2048 (w,a,i,j)]
    xts = []
    for m in range(N_CORES):
        xs = x[:, :, m * HS:(m + 1) * HS, :]                      # (4a, 256, 16, 128)
        t = xs.reshape(N_AGENTS, 2, 128, N_STRIPS, WIN, 32, WIN)  # a c p s i w j
        t = t.transpose(3, 2, 1, 5, 0, 4, 6)                      # s p c w a i j
        xts.append(np.ascontiguousarray(t.reshape(N_STRIPS, 128, 2, ST)).astype(np.float16))
    return xts, wq, wk, wv, wo, bmz, identity


def _unpack_out(res_list):
    out = np.empty((N_AGENTS, DIM, H, W), dtype=np.float32)
    for m in range(N_CORES):
        o = np.asarray(res_list[m]).astype(np.float32)            # (4s, 128p, 4096)
        t = o.reshape(N_STRIPS, 128, 2, N_GROUPS, GW, N_AGENTS, WIN, WIN)
        # -> channel (c*128+p), rows (s, i), cols (g, w8, j)
        t = t.transpose(5, 2, 1, 0, 6, 3, 4, 7)                   # a c p s i g w j
        out[:, :, m * HS:(m + 1) * HS, :] = t.reshape(N_AGENTS, DIM, HS, W)
    return out


def kernel(x, w_qkv, w_out, bias_table, _want_trace=False):
    global _NC_CACHE
    from concourse.bass_utils import run_bass_kernel_spmd

    xts, wq, wk, wv, wo, bmz, identity = _prep_inputs(x, w_qkv, w_out, bias_table)

    if _NC_CACHE is None:
        _NC_CACHE = build_nc()
    nc = _NC_CACHE

    in_maps = []
    for m in range(N_CORES):
        in_maps.append({
            "xt": xts[m], "wq": wq, "wk": wk, "wv": wv, "wo": wo,
            "bm": bmz, "ident": identity,
        })
    res = run_bass_kernel_spmd(nc, in_maps, list(range(N_CORES)), trace=_want_trace)
    out = _unpack_out([res.results[m]["out"] for m in range(N_CORES)])
    if _want_trace:
        return out, res
    return out
